# revision 1
# baseline (speedup 1.0000x reference)
"""Chamfer-loss min/argmin kernel for Trainium2 (8 NeuronCores).

Problem: preds [4, 8192, 3], gts [4, 8192, 3] fp32.
P[b, n, m] = ||gts[b,n]||^2 + ||preds[b,m]||^2 - 2 <gts[b,n], preds[b,m]>
Outputs: (min over n [4,8192], min over m [4,8192],
          argmin over n int32, argmin over m int32).

Sharding: 8 cores = 4 batches x 2 halves of the gts (n) axis. Each core
holds full preds for its batch and a 4096-row slice of gts. Per-gt-row
results (min over m) are final; per-pred-row results (min over n) are
partial over the n-slice and combined on the host.

Device kernel per core (both directions, roles swapped):
 - K=4 fp32 matmuls (rows [-2x0,-2x1,-2x2,1] x [y0,y1,y2,ry]) produce
   Q = -2<x,y> + ry_free directly in PSUM.  Matmuls are quad-packed with
   tile_position row groups (4 concurrent small-K matmuls) filling one
   [128, 2048] PSUM group per quad.
 - DVE tensor_scalar stages each PSUM group to SBUF while adding the
   per-partition norm (P = Q + rx) and min-reducing into a group accum.
 - max_index finds the first-occurrence argmin (jnp tie semantics).
"""

import functools

import numpy as np

BS, N, M, D = 4, 8192, 8192, 3
NSL = N // 2  # gts rows per core
K = 4  # contraction: 3 coords + free-side norm
N_CORES = 8
GROUP = 2048  # PSUM group: 4 banks of 512 fp32


def _build_nc(nsl, m, reps=1):
    import contextlib

    import concourse.bacc as bacc
    import concourse.mybir as mybir
    import concourse.tile as tile

    f32 = mybir.dt.float32
    i32 = mybir.dt.int32
    u32 = mybir.dt.uint32

    nc = bacc.Bacc("TRN2", target_bir_lowering=False, debug=False)

    ga = nc.declare_dram_parameter("ga", [2 * K, nsl], f32, isOutput=False)
    pa = nc.declare_dram_parameter("pa", [2 * K, m], f32, isOutput=False)
    rxc = nc.declare_dram_parameter("rxc", [128, nsl // 128], f32, isOutput=False)
    ryc = nc.declare_dram_parameter("ryc", [128, m // 128], f32, isOutput=False)
    gmin_o = nc.declare_dram_parameter("gmin", [128, nsl // 128 * 8], f32, isOutput=True)
    gidx_o = nc.declare_dram_parameter("gidx", [128, nsl // 128 * 8], i32, isOutput=True)
    pmin_o = nc.declare_dram_parameter("pmin", [128, m // 128 * 8], f32, isOutput=True)
    pidx_o = nc.declare_dram_parameter("pidx", [128, m // 128 * 8], i32, isOutput=True)

    with tile.TileContext(nc) as tc:
        with (
            tc.tile_pool(name="const", bufs=1) as const,
            tc.tile_pool(name="rows", bufs=1) as rows,
            tc.tile_pool(name="small", bufs=4) as small,
            tc.tile_pool(name="outs", bufs=1) as outs,
            tc.tile_pool(name="psum", bufs=2, space="PSUM") as psum,
        ):
            # operands replicated into the 4 PE row groups (partitions 32j,
            # 32-aligned as the matmul requires), one tensor per operand role
            ga_repL = const.tile([128, nsl], f32)
            ga_repR = const.tile([128, nsl], f32)
            pa_repR = const.tile([128, m], f32)
            pa_repL = const.tile([128, m], f32)
            for j in range(4):
                nc.sync.dma_start(ga_repL[32 * j : 32 * j + K, :], ga[0:K, :])
                nc.sync.dma_start(ga_repR[32 * j : 32 * j + K, :], ga[K : 2 * K, :])
                nc.sync.dma_start(pa_repR[32 * j : 32 * j + K, :], pa[0:K, :])
                nc.sync.dma_start(pa_repL[32 * j : 32 * j + K, :], pa[K : 2 * K, :])
            rx_sb = const.tile([128, nsl // 128], f32)
            ry_sb = const.tile([128, m // 128], f32)
            nc.sync.dma_start(rx_sb[:], rxc[:])
            nc.sync.dma_start(ry_sb[:], ryc[:])

            rep_loop = tc.For_i(0, reps, 1) if reps > 1 else contextlib.nullcontext()
            rep_loop.__enter__()

            gmin_sb = outs.tile([128, nsl // 128 * 8], f32)
            gidx_sb = outs.tile([128, nsl // 128 * 8], i32)
            pmin_sb = outs.tile([128, m // 128 * 8], f32)
            pidx_sb = outs.tile([128, m // 128 * 8], i32)

            def direction(n_chunks, lhs_rep, rhs_rep, rhs_len, norm_sb,
                          min_sb, idx_sb, tagp):
                n_groups = rhs_len // GROUP
                # one persistent accumulator tile per direction; pad slots
                # written once by DVE itself (no cross-engine write race)
                tmins = outs.tile([128, 8], f32, tag=f"tmins{tagp}")
                nc.vector.memset(tmins[:, n_groups:8], 3.0e38)
                for ci in range(n_chunks):
                    row = rows.tile([128, rhs_len], f32, tag=f"row{tagp}")
                    for g in range(n_groups):
                        pt = psum.tile([128, GROUP], f32, tag="pt")
                        for j in range(4):
                            lhsT = lhs_rep[32 * j : 32 * j + K,
                                           ci * 128 : (ci + 1) * 128]
                            col0 = g * GROUP + j * 512
                            nc.tensor.matmul(
                                pt[:, j * 512 : (j + 1) * 512],
                                lhsT=lhsT,
                                rhs=rhs_rep[32 * j : 32 * j + K, col0 : col0 + 512],
                                start=True,
                                stop=True,
                                tile_position=(32 * j, 0),
                            )
                        nc.vector.tensor_scalar(
                            row[:, g * GROUP : (g + 1) * GROUP],
                            pt[:],
                            norm_sb[:, ci : ci + 1],
                            None,
                            op0=mybir.AluOpType.add,
                            op1=mybir.AluOpType.min,
                            accum_out=tmins[:, g : g + 1],
                        )
                    # max_index searches the 8 group accumulators directly:
                    # the winning slot's first-occurrence IS the argmin; the
                    # slot is selected on the host from the shipped values.
                    i8 = small.tile([128, 8], u32, tag=f"i8{tagp}")
                    nc.vector.max_index(out=i8[:], in_max=tmins[:], in_values=row[:])
                    nc.scalar.activation(
                        min_sb[:, ci * 8 : (ci + 1) * 8], tmins[:],
                        mybir.ActivationFunctionType.Identity,
                    )
                    nc.gpsimd.tensor_copy(
                        out=idx_sb[:, ci * 8 : (ci + 1) * 8], in_=i8[:]
                    )

            # per-gt rows: min/argmin over preds (final)
            direction(nsl // 128, ga_repL, pa_repR, m, rx_sb, gmin_sb, gidx_sb, "g")
            # per-pred rows: min/argmin over the gts slice (partial)
            direction(m // 128, pa_repL, ga_repR, nsl, ry_sb, pmin_sb, pidx_sb, "p")

            nc.sync.dma_start(gmin_o[:], gmin_sb[:])
            nc.sync.dma_start(gidx_o[:], gidx_sb[:])
            nc.sync.dma_start(pmin_o[:], pmin_sb[:])
            nc.sync.dma_start(pidx_o[:], pidx_sb[:])

            rep_loop.__exit__(None, None, None)
    nc.finalize()
    return nc


@functools.lru_cache(maxsize=None)
def _get_nc(nsl, m, reps=1):
    return _build_nc(nsl, m, reps)


def _augment(preds_b, gts_bh):
    """Operands for the K=4 scheme.

    ga rows: [-2x0, -2x1, -2x2, 1]  (gts slice, [4, nsl])
    pa rows: [y0, y1, y2, ry]       (full preds, [4, m])
    matmul: Q[n, m] = -2<x_n, y_m> + ry[m];  P = Q + rx via tensor_scalar.
    rxc/ryc: norms laid out [128, len//128] column-per-chunk.
    """
    x = np.ascontiguousarray(gts_bh, dtype=np.float32)
    y = np.ascontiguousarray(preds_b, dtype=np.float32)
    nsl = x.shape[0]
    m = y.shape[0]
    rx = (x[:, 0] * x[:, 0] + x[:, 1] * x[:, 1] + x[:, 2] * x[:, 2]).astype(np.float32)
    ry = (y[:, 0] * y[:, 0] + y[:, 1] * y[:, 1] + y[:, 2] * y[:, 2]).astype(np.float32)
    ga = np.empty((2 * K, nsl), np.float32)
    ga[0:3] = (np.float32(-2.0) * x).T   # lhsT rows, per-gt
    ga[3] = 1.0
    ga[4:7] = ga[0:3]                    # rhs rows, per-pred
    ga[7] = rx
    pa = np.empty((2 * K, m), np.float32)
    pa[0:3] = y.T                        # rhs rows, per-gt
    pa[3] = ry
    pa[4:7] = y.T                        # lhsT rows, per-pred
    pa[7] = 1.0
    rxc = np.ascontiguousarray(rx.reshape(nsl // 128, 128).T)
    ryc = np.ascontiguousarray(ry.reshape(m // 128, 128).T)
    return ga, pa, rxc, ryc


@functools.lru_cache(maxsize=None)
def _get_dispatcher(nsl, m, reps=1):
    """Build the SPMD PJRT dispatcher once and cache it (the stock
    run_bass_via_pjrt re-traces jax.jit on every call)."""
    import jax
    import numpy as _np
    from jax.sharding import Mesh, PartitionSpec
    from jax.experimental.shard_map import shard_map
    import concourse.mybir as mybir
    from concourse import bass2jax

    bass2jax.install_neuronx_cc_hook()
    nc = _get_nc(nsl, m, reps)

    partition_name = nc.partition_id_tensor.name if nc.partition_id_tensor else None
    in_names, out_names, out_avals, zero_outs = [], [], [], []
    for alloc in nc.m.functions[0].allocations:
        if not isinstance(alloc, mybir.MemoryLocationSet):
            continue
        name = alloc.memorylocations[0].name
        if alloc.kind == "ExternalInput":
            if name != partition_name:
                in_names.append(name)
        elif alloc.kind == "ExternalOutput":
            shape = tuple(alloc.tensor_shape)
            dtype = mybir.dt.np(alloc.dtype)
            out_names.append(name)
            out_avals.append(jax.core.ShapedArray(shape, dtype))
            zero_outs.append(_np.zeros(shape, dtype))
    n_params = len(in_names)
    n_outs = len(out_avals)
    all_in_names = list(in_names) + list(out_names)
    if partition_name is not None:
        all_in_names.append(partition_name)
    donate = tuple(range(n_params, n_params + n_outs))

    def _body(*args):
        operands = list(args)
        if partition_name is not None:
            operands.append(bass2jax.partition_id_tensor())
        outs = bass2jax._bass_exec_p.bind(
            *operands,
            out_avals=tuple(out_avals),
            in_names=tuple(all_in_names),
            out_names=tuple(out_names),
            lowering_input_output_aliases=(),
            sim_require_finite=True,
            sim_require_nnan=True,
            nc=nc,
        )
        return tuple(outs)

    devices = jax.devices()[:N_CORES]
    mesh = Mesh(np.asarray(devices), ("core",))
    in_specs = (PartitionSpec("core"),) * (n_params + n_outs)
    out_specs = (PartitionSpec("core"),) * n_outs
    sharded = jax.jit(
        shard_map(_body, mesh=mesh, in_specs=in_specs, out_specs=out_specs,
                  check_rep=False),
        donate_argnums=donate,
        keep_unused=True,
    )

    def dispatch(in_maps):
        concat_in = [
            np.concatenate([np.asarray(in_maps[c][nm]) for c in range(N_CORES)], axis=0)
            for nm in in_names
        ]
        concat_zeros = [
            np.zeros((N_CORES * z.shape[0], *z.shape[1:]), z.dtype) for z in zero_outs
        ]
        out_arrs = sharded(*concat_in, *concat_zeros)
        return [
            {nm: np.asarray(out_arrs[i]).reshape(N_CORES, *out_avals[i].shape)[c]
             for i, nm in enumerate(out_names)}
            for c in range(N_CORES)
        ]

    return dispatch


def _make_in_maps(preds, gts):
    in_maps = []
    for c in range(N_CORES):
        b, h = c // 2, c % 2
        ga, pa, rxc, ryc = _augment(preds[b], gts[b, h * NSL : (h + 1) * NSL])
        in_maps.append({"ga": ga, "pa": pa, "rxc": rxc, "ryc": ryc})
    return in_maps


def kernel(preds, gts, mask):
    preds = np.asarray(preds, dtype=np.float32)
    gts = np.asarray(gts, dtype=np.float32)

    results = _get_dispatcher(NSL, M)(_make_in_maps(preds, gts))

    out_pmin = np.empty((BS, M), np.float32)
    out_gmin = np.empty((BS, N), np.float32)
    out_pidx = np.empty((BS, M), np.int32)
    out_gidx = np.empty((BS, N), np.int32)

    def _pick(r, key_min, key_idx, length):
        tm = (r[key_min].reshape(128, length // 128, 8)
              .transpose(1, 0, 2).reshape(length, 8))
        ii = (r[key_idx].reshape(128, length // 128, 8)
              .transpose(1, 0, 2).reshape(length, 8))
        js = tm.argmin(1)
        rows_ = np.arange(length)
        return tm[rows_, js], ii[rows_, js]

    for b in range(BS):
        r0, r1 = results[2 * b], results[2 * b + 1]
        # per-gt rows (min over preds): each half is final
        for h, r in ((0, r0), (1, r1)):
            gm, gi = _pick(r, "gmin", "gidx", NSL)
            out_gmin[b, h * NSL : (h + 1) * NSL] = gm
            out_gidx[b, h * NSL : (h + 1) * NSL] = gi
        # per-pred rows: combine the two n-halves
        pm0, pi0 = _pick(r0, "pmin", "pidx", M)
        pm1, pi1 = _pick(r1, "pmin", "pidx", M)
        take1 = pm1 < pm0  # tie -> half 0 (lower gt index), first occurrence
        out_pmin[b] = np.where(take1, pm1, pm0)
        out_pidx[b] = np.where(take1, pi1 + NSL, pi0)

    return out_pmin, out_gmin, out_pidx, out_gidx



# revision 2
# speedup vs baseline: 1.4542x; 1.4542x over previous
"""Chamfer-loss min/argmin kernel for Trainium2 (8 NeuronCores).

Problem: preds [4, 8192, 3], gts [4, 8192, 3] fp32.
P[b, n, m] = ||gts[b,n]||^2 + ||preds[b,m]||^2 - 2 <gts[b,n], preds[b,m]>
Outputs: (min over n [4,8192], min over m [4,8192],
          argmin over n int32, argmin over m int32).

Sharding: 8 cores = 4 batches x 2 halves of the gts (n) axis. Each core
holds full preds for its batch and a 4096-row slice of gts.

Device kernel per core (both directions, roles swapped):
 - K=5 fp32 matmuls produce G = -P directly in PSUM (rows
   [2x0,2x1,2x2,-1,-rx] x [y0,y1,y2,ry,1]); quad-packed with
   tile_position row groups filling one [128, 2048] PSUM group.
 - ONE custom DVE op per group streams the PSUM group in REVERSE and
   computes accum = max stream-idx where elem == running-max.  On the
   reversed stream that is the first-occurrence argmax of G = argmin of
   P within the group.  No value pass, no max_index pass: the min
   values are reconstructed on the host from the winning indices.
"""

import functools

import numpy as np

BS, N, M, D = 4, 8192, 8192, 3
NSL = N // 2  # gts rows per core
K = 5  # contraction: 3 coords + both norms
N_CORES = 8
GROUP = 2048  # PSUM group: 4 banks of 512 fp32
FMAX_NEG = np.float32(-3.4028234663852886e38)


def _register_argmax_op():
    """Register the RUNMAX_LAST_IDX_ANT custom DVE op (runtime append to
    dve_ops.OPS — the documented extension point; kernel.py must be
    self-contained so it cannot edit dve_ops.py)."""
    from concourse import dve_ops as dops
    from concourse.dve_spec import (
        Spec, Src0, MaxNeg, AluOp, eq, select, scan, Idx,
        lower as dve_lower, _has_src1,
    )
    from concourse.dve_uop import DveOpSpec

    name = "RUNMAX_LAST_IDX_ANT"
    for o in dops.OPS:
        if o.name == name:
            return o

    r = scan(AluOp.MAX, Src0)
    body = select(eq(Src0, r), Idx, MaxNeg)

    def _ref(in0, in1, s0, s1, imm2):
        x = np.asarray(in0, np.float32)
        xf = x.reshape(x.shape[0], -1)
        rm = np.maximum.accumulate(xf, axis=-1)
        idx = np.arange(xf.shape[-1], dtype=np.float32)
        out = np.where(xf == rm, idx, FMAX_NEG)
        acc = out.max(axis=-1, keepdims=True)
        return out.reshape(x.shape).astype(np.float32), acc.astype(np.float32)

    spec = Spec(body=body, accum=AluOp.MAX, reference=_ref)
    shas = {}
    for ver in ("v3", "v4"):
        s = DveOpSpec(name=name, opcode=1, uops=dve_lower(spec, ver=ver),
                      rd1_en=_has_src1(spec))
        shas[ver] = s.sha(ver)
    op = dops.DveOp(name, spec, subdim=False, uops_sha=shas)
    dops.OPS.append(op)
    dops.CUSTOM_DVE_SPECS[name] = spec
    dops._SUB_OPCODE_FOR_NAME[name] = max(dops._SUB_OPCODE_FOR_NAME.values()) + 1
    return op


def _build_nc(nsl, m, reps=1):
    import contextlib

    import concourse.bacc as bacc
    import concourse.mybir as mybir
    import concourse.tile as tile

    op = _register_argmax_op()
    f32 = mybir.dt.float32

    nc = bacc.Bacc("TRN2", target_bir_lowering=False, debug=False)

    ga = nc.declare_dram_parameter("ga", [2 * K, nsl], f32, isOutput=False)
    pa = nc.declare_dram_parameter("pa", [2 * K, m], f32, isOutput=False)
    ng1 = m // GROUP     # groups per dir-1 chunk
    ng2 = nsl // GROUP   # groups per dir-2 chunk
    gacc_o = nc.declare_dram_parameter("gacc", [128, nsl // 128 * ng1], f32,
                                       isOutput=True)
    pacc_o = nc.declare_dram_parameter("pacc", [128, m // 128 * ng2], f32,
                                       isOutput=True)

    with tile.TileContext(nc) as tc:
        with (
            tc.tile_pool(name="const", bufs=1) as const,
            tc.tile_pool(name="scr", bufs=1) as scr,
            tc.tile_pool(name="outs", bufs=1) as outs,
            tc.tile_pool(name="psum", bufs=2, space="PSUM") as psum,
        ):
            # operands replicated into the 4 PE row groups (partitions 32j)
            ga_repL = const.tile([128, nsl], f32)
            ga_repR = const.tile([128, nsl], f32)
            pa_repR = const.tile([128, m], f32)
            pa_repL = const.tile([128, m], f32)
            for j in range(4):
                nc.sync.dma_start(ga_repL[32 * j : 32 * j + K, :], ga[0:K, :])
                nc.sync.dma_start(ga_repR[32 * j : 32 * j + K, :], ga[K : 2 * K, :])
                nc.sync.dma_start(pa_repR[32 * j : 32 * j + K, :], pa[0:K, :])
                nc.sync.dma_start(pa_repL[32 * j : 32 * j + K, :], pa[K : 2 * K, :])

            rep_loop = tc.For_i(0, reps, 1) if reps > 1 else contextlib.nullcontext()
            rep_loop.__enter__()

            gacc_sb = outs.tile([128, nsl // 128 * ng1], f32)
            pacc_sb = outs.tile([128, m // 128 * ng2], f32)
            scratch = scr.tile([128, GROUP], f32)

            def direction(n_chunks, lhs_rep, rhs_rep, rhs_len, acc_sb, tagp):
                n_groups = rhs_len // GROUP
                for ci in range(n_chunks):
                    for g in range(n_groups):
                        pt = psum.tile([128, GROUP], f32, tag="pt")
                        for j in range(4):
                            lhsT = lhs_rep[32 * j : 32 * j + K,
                                           ci * 128 : (ci + 1) * 128]
                            col0 = g * GROUP + j * 512
                            nc.tensor.matmul(
                                pt[:, j * 512 : (j + 1) * 512],
                                lhsT=lhsT,
                                rhs=rhs_rep[32 * j : 32 * j + K, col0 : col0 + 512],
                                start=True,
                                stop=True,
                                tile_position=(32 * j, 0),
                            )
                        nc.vector._custom_dve(
                            op,
                            out=scratch[:],
                            in0=pt[:, ::-1],
                            accum_out=acc_sb[:, ci * n_groups + g :
                                             ci * n_groups + g + 1],
                        )

            # per-gt rows: argmin over preds (final)
            direction(nsl // 128, ga_repL, pa_repR, m, gacc_sb, "g")
            # per-pred rows: argmin over the gts slice (partial)
            direction(m // 128, pa_repL, ga_repR, nsl, pacc_sb, "p")

            nc.sync.dma_start(gacc_o[:], gacc_sb[:])
            nc.sync.dma_start(pacc_o[:], pacc_sb[:])

            rep_loop.__exit__(None, None, None)
    nc.finalize()
    return nc


@functools.lru_cache(maxsize=None)
def _get_nc(nsl, m, reps=1):
    return _build_nc(nsl, m, reps)


def _augment(preds_b, gts_bh):
    """Operands for the negated K=5 scheme (G = -P in PSUM).

    ga rows 0-4 (dir-1 lhsT): [2x0, 2x1, 2x2, -1, -rx]
    ga rows 5-9 (dir-2 rhs):  [x0, x1, x2, rx, 1]
    pa rows 0-4 (dir-1 rhs):  [y0, y1, y2, ry, 1]
    pa rows 5-9 (dir-2 lhsT): [2y0, 2y1, 2y2, -1, -ry]
    G[n,m] = 2<x,y> - ry - rx = -P.
    """
    x = np.ascontiguousarray(gts_bh, dtype=np.float32)
    y = np.ascontiguousarray(preds_b, dtype=np.float32)
    nsl = x.shape[0]
    m = y.shape[0]
    rx = (x[:, 0] * x[:, 0] + x[:, 1] * x[:, 1] + x[:, 2] * x[:, 2]).astype(np.float32)
    ry = (y[:, 0] * y[:, 0] + y[:, 1] * y[:, 1] + y[:, 2] * y[:, 2]).astype(np.float32)
    ga = np.empty((2 * K, nsl), np.float32)
    ga[0:3] = np.float32(2.0) * x.T
    ga[3] = -1.0
    ga[4] = -rx
    ga[5:8] = x.T
    ga[8] = rx
    ga[9] = 1.0
    pa = np.empty((2 * K, m), np.float32)
    pa[0:3] = y.T
    pa[3] = ry
    pa[4] = 1.0
    pa[5:8] = np.float32(2.0) * y.T
    pa[8] = -1.0
    pa[9] = -ry
    return ga, pa


@functools.lru_cache(maxsize=None)
def _get_dispatcher(nsl, m, reps=1):
    """Build the SPMD PJRT dispatcher once and cache it (the stock
    run_bass_via_pjrt re-traces jax.jit on every call)."""
    import jax
    import numpy as _np
    from jax.sharding import Mesh, PartitionSpec
    from jax.experimental.shard_map import shard_map
    import concourse.mybir as mybir
    from concourse import bass2jax

    bass2jax.install_neuronx_cc_hook()
    nc = _get_nc(nsl, m, reps)

    partition_name = nc.partition_id_tensor.name if nc.partition_id_tensor else None
    in_names, out_names, out_avals, zero_outs = [], [], [], []
    for alloc in nc.m.functions[0].allocations:
        if not isinstance(alloc, mybir.MemoryLocationSet):
            continue
        name = alloc.memorylocations[0].name
        if alloc.kind == "ExternalInput":
            if name != partition_name:
                in_names.append(name)
        elif alloc.kind == "ExternalOutput":
            shape = tuple(alloc.tensor_shape)
            dtype = mybir.dt.np(alloc.dtype)
            out_names.append(name)
            out_avals.append(jax.core.ShapedArray(shape, dtype))
            zero_outs.append(_np.zeros(shape, dtype))
    n_params = len(in_names)
    n_outs = len(out_avals)
    all_in_names = list(in_names) + list(out_names)
    if partition_name is not None:
        all_in_names.append(partition_name)
    donate = tuple(range(n_params, n_params + n_outs))

    def _body(*args):
        operands = list(args)
        if partition_name is not None:
            operands.append(bass2jax.partition_id_tensor())
        outs = bass2jax._bass_exec_p.bind(
            *operands,
            out_avals=tuple(out_avals),
            in_names=tuple(all_in_names),
            out_names=tuple(out_names),
            lowering_input_output_aliases=(),
            sim_require_finite=True,
            sim_require_nnan=True,
            nc=nc,
        )
        return tuple(outs)

    devices = jax.devices()[:N_CORES]
    mesh = Mesh(np.asarray(devices), ("core",))
    in_specs = (PartitionSpec("core"),) * (n_params + n_outs)
    out_specs = (PartitionSpec("core"),) * n_outs
    sharded = jax.jit(
        shard_map(_body, mesh=mesh, in_specs=in_specs, out_specs=out_specs,
                  check_rep=False),
        donate_argnums=donate,
        keep_unused=True,
    )

    def dispatch(in_maps):
        concat_in = [
            np.concatenate([np.asarray(in_maps[c][nm]) for c in range(N_CORES)], axis=0)
            for nm in in_names
        ]
        concat_zeros = [
            np.zeros((N_CORES * z.shape[0], *z.shape[1:]), z.dtype) for z in zero_outs
        ]
        out_arrs = sharded(*concat_in, *concat_zeros)
        return [
            {nm: np.asarray(out_arrs[i]).reshape(N_CORES, *out_avals[i].shape)[c]
             for i, nm in enumerate(out_names)}
            for c in range(N_CORES)
        ]

    return dispatch


def _make_in_maps(preds, gts):
    in_maps = []
    for c in range(N_CORES):
        b, h = c // 2, c % 2
        ga, pa = _augment(preds[b], gts[b, h * NSL : (h + 1) * NSL])
        in_maps.append({"ga": ga, "pa": pa})
    return in_maps


def _decode_candidates(acc, n_rows, n_groups):
    """acc: [128, n_rows//128 * n_groups] reversed-stream indices ->
    candidate column indices [n_rows, n_groups] (ascending per row)."""
    k = np.rint(np.asarray(acc, np.float64)).astype(np.int64)
    k = k.reshape(128, n_rows // 128, n_groups).transpose(1, 0, 2)
    k = k.reshape(n_rows, n_groups)
    local = (GROUP - 1) - k
    return local + np.arange(n_groups, dtype=np.int64)[None, :] * GROUP


def kernel(preds, gts, mask):
    preds = np.asarray(preds, dtype=np.float32)
    gts = np.asarray(gts, dtype=np.float32)

    results = _get_dispatcher(NSL, M)(_make_in_maps(preds, gts))

    out_pmin = np.empty((BS, M), np.float32)
    out_gmin = np.empty((BS, N), np.float32)
    out_pidx = np.empty((BS, M), np.int32)
    out_gidx = np.empty((BS, N), np.int32)

    ng1 = M // GROUP
    ng2 = NSL // GROUP

    for b in range(BS):
        r0, r1 = results[2 * b], results[2 * b + 1]
        xb = gts[b].astype(np.float64)
        yb = preds[b].astype(np.float64)

        # per-gt rows (min over preds): each half is final
        for h, r in ((0, r0), (1, r1)):
            cand = _decode_candidates(r["gacc"], NSL, ng1)  # [NSL, 4] m-indices
            xrows = xb[h * NSL : (h + 1) * NSL]             # [NSL, 3]
            diff = xrows[:, None, :] - yb[cand]             # [NSL, 4, 3]
            d = np.einsum("ngd,ngd->ng", diff, diff)        # [NSL, 4]
            j = np.argmin(d, axis=1)                        # ties -> lower m
            rows = np.arange(NSL)
            out_gmin[b, h * NSL : (h + 1) * NSL] = d[rows, j].astype(np.float32)
            out_gidx[b, h * NSL : (h + 1) * NSL] = cand[rows, j].astype(np.int32)

        # per-pred rows: combine the two n-halves (candidates ascending in n)
        c0 = _decode_candidates(r0["pacc"], M, ng2)          # [M, 2] in half 0
        c1 = _decode_candidates(r1["pacc"], M, ng2) + NSL    # [M, 2] in half 1
        cand = np.concatenate([c0, c1], axis=1)              # [M, 4] ascending
        diff = yb[:, None, :] - xb[cand]                     # [M, 4, 3]
        d = np.einsum("mgd,mgd->mg", diff, diff)
        j = np.argmin(d, axis=1)                             # ties -> lower n
        rows = np.arange(M)
        out_pmin[b] = d[rows, j].astype(np.float32)
        out_pidx[b] = cand[rows, j].astype(np.int32)

    return out_pmin, out_gmin, out_pidx, out_gidx


# revision 7
# speedup vs baseline: 1.6619x; 1.1428x over previous
"""Chamfer-loss min/argmin kernel for Trainium2 (8 NeuronCores).

Problem: preds [4, 8192, 3], gts [4, 8192, 3] fp32.
P[b, n, m] = ||gts[b,n]||^2 + ||preds[b,m]||^2 - 2 <gts[b,n], preds[b,m]>
Outputs: (min over n [4,8192], min over m [4,8192],
          argmin over n int32, argmin over m int32).

Sharding: 8 cores = 4 batches x 2 halves of the gts (n) axis. Each core
holds full preds for its batch and a 4096-row slice of gts.

Device kernel per core (both directions, roles swapped):
 - K=5 fp32 matmuls produce G = -P directly in PSUM (rows
   [2x0,2x1,2x2,-1,-rx] x [y0,y1,y2,ry,1]); quad-packed with
   tile_position row groups filling one [128, 2048] PSUM group.
 - ONE custom DVE op per group streams the PSUM group in REVERSE and
   computes accum = max stream-idx where elem == running-max.  On the
   reversed stream that is the first-occurrence argmax of G = argmin of
   P within the group.  No value pass, no max_index pass: the min
   values are reconstructed on the host from the winning indices.
"""

import functools

import numpy as np

BS, N, M, D = 4, 8192, 8192, 3
NSL = N // 2  # gts rows per core
K = 5  # contraction: 3 coords + both norms
N_CORES = 8
GROUP = 2048  # PSUM group: 4 banks of 512 fp32
FMAX_NEG = np.float32(-3.4028234663852886e38)


def _register_argmax_op():
    """Register the RUNMAX_LAST_IDX_ANT custom DVE op (runtime append to
    dve_ops.OPS — the documented extension point; kernel.py must be
    self-contained so it cannot edit dve_ops.py)."""
    from concourse import dve_ops as dops
    from concourse.dve_spec import (
        Spec, Src0, MaxNeg, AluOp, eq, select, scan, Idx,
        lower as dve_lower, _has_src1,
    )
    from concourse.dve_uop import DveOpSpec

    name = "RUNMAX_LAST_IDX_ANT"
    for o in dops.OPS:
        if o.name == name:
            return o

    r = scan(AluOp.MAX, Src0)
    body = select(eq(Src0, r), Idx, MaxNeg)

    def _ref(in0, in1, s0, s1, imm2):
        x = np.asarray(in0, np.float32)
        xf = x.reshape(x.shape[0], -1)
        rm = np.maximum.accumulate(xf, axis=-1)
        idx = np.arange(xf.shape[-1], dtype=np.float32)
        out = np.where(xf == rm, idx, FMAX_NEG)
        acc = out.max(axis=-1, keepdims=True)
        return out.reshape(x.shape).astype(np.float32), acc.astype(np.float32)

    spec = Spec(body=body, accum=AluOp.MAX, reference=_ref)
    shas = {}
    for ver in ("v3", "v4"):
        s = DveOpSpec(name=name, opcode=1, uops=dve_lower(spec, ver=ver),
                      rd1_en=_has_src1(spec))
        shas[ver] = s.sha(ver)
    op = dops.DveOp(name, spec, subdim=False, uops_sha=shas)
    dops.OPS.append(op)
    dops.CUSTOM_DVE_SPECS[name] = spec
    dops._SUB_OPCODE_FOR_NAME[name] = max(dops._SUB_OPCODE_FOR_NAME.values()) + 1
    return op


def _build_nc(nsl, m, reps=1, scratch_bufs=1, reverse=True):
    import contextlib

    import concourse.bacc as bacc
    import concourse.mybir as mybir
    import concourse.tile as tile

    op = _register_argmax_op()
    f32 = mybir.dt.float32

    nc = bacc.Bacc("TRN2", target_bir_lowering=False, debug=False)

    ga = nc.declare_dram_parameter("ga", [2 * K, nsl], f32, isOutput=False)
    pa = nc.declare_dram_parameter("pa", [2 * K, m], f32, isOutput=False)
    ng1 = m // GROUP     # groups per dir-1 chunk
    ng2 = nsl // GROUP   # groups per dir-2 chunk
    gacc_o = nc.declare_dram_parameter("gacc", [128, nsl // 128 * ng1], f32,
                                       isOutput=True)
    pacc_o = nc.declare_dram_parameter("pacc", [128, m // 128 * ng2], f32,
                                       isOutput=True)

    with tile.TileContext(nc) as tc:
        with (
            tc.tile_pool(name="const", bufs=1) as const,
            tc.tile_pool(name="scr", bufs=scratch_bufs) as scr,
            tc.tile_pool(name="outs", bufs=1) as outs,
            tc.tile_pool(name="psum", bufs=2, space="PSUM") as psum,
        ):
            # operands replicated into the 4 PE row groups (partitions 32j)
            ga_repL = const.tile([128, nsl], f32)
            ga_repR = const.tile([128, nsl], f32)
            pa_repR = const.tile([128, m], f32)
            pa_repL = const.tile([128, m], f32)
            for j in range(4):
                nc.sync.dma_start(ga_repL[32 * j : 32 * j + K, :], ga[0:K, :])
                nc.sync.dma_start(ga_repR[32 * j : 32 * j + K, :], ga[K : 2 * K, :])
                nc.sync.dma_start(pa_repR[32 * j : 32 * j + K, :], pa[0:K, :])
                nc.sync.dma_start(pa_repL[32 * j : 32 * j + K, :], pa[K : 2 * K, :])

            rep_loop = tc.For_i(0, reps, 1) if reps > 1 else contextlib.nullcontext()
            rep_loop.__enter__()

            gacc_sb = outs.tile([128, nsl // 128 * ng1], f32)
            pacc_sb = outs.tile([128, m // 128 * ng2], f32)

            def direction(n_chunks, lhs_rep, rhs_rep, rhs_len, acc_sb, tagp):
                n_groups = rhs_len // GROUP
                for ci in range(n_chunks):
                    for g in range(n_groups):
                        pt = psum.tile([128, GROUP], f32, tag="pt")
                        for j in range(4):
                            lhsT = lhs_rep[32 * j : 32 * j + K,
                                           ci * 128 : (ci + 1) * 128]
                            col0 = g * GROUP + j * 512
                            nc.tensor.matmul(
                                pt[:, j * 512 : (j + 1) * 512],
                                lhsT=lhsT,
                                rhs=rhs_rep[32 * j : 32 * j + K, col0 : col0 + 512],
                                start=True,
                                stop=True,
                                tile_position=(32 * j, 0),
                            )
                        scratch = scr.tile([128, GROUP], f32, tag="scratch")
                        nc.vector._custom_dve(
                            op,
                            out=scratch[:],
                            in0=pt[:, ::-1] if reverse else pt[:],
                            accum_out=acc_sb[:, ci * n_groups + g :
                                             ci * n_groups + g + 1],
                        )

            # per-gt rows: argmin over preds (final)
            direction(nsl // 128, ga_repL, pa_repR, m, gacc_sb, "g")
            # per-pred rows: argmin over the gts slice (partial)
            direction(m // 128, pa_repL, ga_repR, nsl, pacc_sb, "p")

            nc.sync.dma_start(gacc_o[:], gacc_sb[:])
            nc.sync.dma_start(pacc_o[:], pacc_sb[:])

            rep_loop.__exit__(None, None, None)
    nc.finalize()
    return nc


@functools.lru_cache(maxsize=None)
def _get_nc(nsl, m, reps=1, scratch_bufs=1, reverse=True):
    return _build_nc(nsl, m, reps, scratch_bufs, reverse)


def _augment(preds_b, gts_bh):
    """Operands for the negated K=5 scheme (G = -P in PSUM).

    ga rows 0-4 (dir-1 lhsT): [2x0, 2x1, 2x2, -1, -rx]
    ga rows 5-9 (dir-2 rhs):  [x0, x1, x2, rx, 1]
    pa rows 0-4 (dir-1 rhs):  [y0, y1, y2, ry, 1]
    pa rows 5-9 (dir-2 lhsT): [2y0, 2y1, 2y2, -1, -ry]
    G[n,m] = 2<x,y> - ry - rx = -P.
    """
    x = np.ascontiguousarray(gts_bh, dtype=np.float32)
    y = np.ascontiguousarray(preds_b, dtype=np.float32)
    nsl = x.shape[0]
    m = y.shape[0]
    rx = (x[:, 0] * x[:, 0] + x[:, 1] * x[:, 1] + x[:, 2] * x[:, 2]).astype(np.float32)
    ry = (y[:, 0] * y[:, 0] + y[:, 1] * y[:, 1] + y[:, 2] * y[:, 2]).astype(np.float32)
    ga = np.empty((2 * K, nsl), np.float32)
    ga[0:3] = np.float32(2.0) * x.T
    ga[3] = -1.0
    ga[4] = -rx
    ga[5:8] = x.T
    ga[8] = rx
    ga[9] = 1.0
    pa = np.empty((2 * K, m), np.float32)
    pa[0:3] = y.T
    pa[3] = ry
    pa[4] = 1.0
    pa[5:8] = np.float32(2.0) * y.T
    pa[8] = -1.0
    pa[9] = -ry
    return ga, pa


@functools.lru_cache(maxsize=None)
def _get_dispatcher(nsl, m, reps=1, scratch_bufs=1, reverse=True):
    """Build the SPMD PJRT dispatcher once and cache it (the stock
    run_bass_via_pjrt re-traces jax.jit on every call)."""
    import jax
    import numpy as _np
    from jax.sharding import Mesh, PartitionSpec
    from jax.experimental.shard_map import shard_map
    import concourse.mybir as mybir
    from concourse import bass2jax

    bass2jax.install_neuronx_cc_hook()
    nc = _get_nc(nsl, m, reps, scratch_bufs, reverse)

    partition_name = nc.partition_id_tensor.name if nc.partition_id_tensor else None
    in_names, out_names, out_avals, zero_outs = [], [], [], []
    for alloc in nc.m.functions[0].allocations:
        if not isinstance(alloc, mybir.MemoryLocationSet):
            continue
        name = alloc.memorylocations[0].name
        if alloc.kind == "ExternalInput":
            if name != partition_name:
                in_names.append(name)
        elif alloc.kind == "ExternalOutput":
            shape = tuple(alloc.tensor_shape)
            dtype = mybir.dt.np(alloc.dtype)
            out_names.append(name)
            out_avals.append(jax.core.ShapedArray(shape, dtype))
            zero_outs.append(_np.zeros(shape, dtype))
    n_params = len(in_names)
    n_outs = len(out_avals)
    all_in_names = list(in_names) + list(out_names)
    if partition_name is not None:
        all_in_names.append(partition_name)
    donate = tuple(range(n_params, n_params + n_outs))

    def _body(*args):
        operands = list(args)
        if partition_name is not None:
            operands.append(bass2jax.partition_id_tensor())
        outs = bass2jax._bass_exec_p.bind(
            *operands,
            out_avals=tuple(out_avals),
            in_names=tuple(all_in_names),
            out_names=tuple(out_names),
            lowering_input_output_aliases=(),
            sim_require_finite=True,
            sim_require_nnan=True,
            nc=nc,
        )
        return tuple(outs)

    devices = jax.devices()[:N_CORES]
    mesh = Mesh(np.asarray(devices), ("core",))
    in_specs = (PartitionSpec("core"),) * (n_params + n_outs)
    out_specs = (PartitionSpec("core"),) * n_outs
    sharded = jax.jit(
        shard_map(_body, mesh=mesh, in_specs=in_specs, out_specs=out_specs,
                  check_rep=False),
        donate_argnums=donate,
        keep_unused=True,
    )

    def dispatch(in_maps):
        concat_in = [
            np.concatenate([np.asarray(in_maps[c][nm]) for c in range(N_CORES)], axis=0)
            for nm in in_names
        ]
        concat_zeros = [
            np.zeros((N_CORES * z.shape[0], *z.shape[1:]), z.dtype) for z in zero_outs
        ]
        out_arrs = sharded(*concat_in, *concat_zeros)
        return [
            {nm: np.asarray(out_arrs[i]).reshape(N_CORES, *out_avals[i].shape)[c]
             for i, nm in enumerate(out_names)}
            for c in range(N_CORES)
        ]

    return dispatch


def _make_in_maps(preds, gts):
    in_maps = []
    for c in range(N_CORES):
        b, h = c // 2, c % 2
        ga, pa = _augment(preds[b], gts[b, h * NSL : (h + 1) * NSL])
        in_maps.append({"ga": ga, "pa": pa})
    return in_maps


def _decode_candidates(acc, n_rows, n_groups):
    """acc: [128, n_rows//128 * n_groups] reversed-stream indices ->
    candidate column indices [n_rows, n_groups] (ascending per row)."""
    k = np.rint(np.asarray(acc, np.float64)).astype(np.int64)
    k = k.reshape(128, n_rows // 128, n_groups).transpose(1, 0, 2)
    k = k.reshape(n_rows, n_groups)
    local = (GROUP - 1) - k
    return local + np.arange(n_groups, dtype=np.int64)[None, :] * GROUP


def kernel(preds, gts, mask):
    preds = np.asarray(preds, dtype=np.float32)
    gts = np.asarray(gts, dtype=np.float32)

    results = _get_dispatcher(NSL, M)(_make_in_maps(preds, gts))

    out_pmin = np.empty((BS, M), np.float32)
    out_gmin = np.empty((BS, N), np.float32)
    out_pidx = np.empty((BS, M), np.int32)
    out_gidx = np.empty((BS, N), np.int32)

    ng1 = M // GROUP
    ng2 = NSL // GROUP

    for b in range(BS):
        r0, r1 = results[2 * b], results[2 * b + 1]
        xb = gts[b].astype(np.float64)
        yb = preds[b].astype(np.float64)

        # per-gt rows (min over preds): each half is final
        for h, r in ((0, r0), (1, r1)):
            cand = _decode_candidates(r["gacc"], NSL, ng1)  # [NSL, 4] m-indices
            xrows = xb[h * NSL : (h + 1) * NSL]             # [NSL, 3]
            diff = xrows[:, None, :] - yb[cand]             # [NSL, 4, 3]
            d = np.einsum("ngd,ngd->ng", diff, diff)        # [NSL, 4]
            j = np.argmin(d, axis=1)                        # ties -> lower m
            rows = np.arange(NSL)
            out_gmin[b, h * NSL : (h + 1) * NSL] = d[rows, j].astype(np.float32)
            out_gidx[b, h * NSL : (h + 1) * NSL] = cand[rows, j].astype(np.int32)

        # per-pred rows: combine the two n-halves (candidates ascending in n)
        c0 = _decode_candidates(r0["pacc"], M, ng2)          # [M, 2] in half 0
        c1 = _decode_candidates(r1["pacc"], M, ng2) + NSL    # [M, 2] in half 1
        cand = np.concatenate([c0, c1], axis=1)              # [M, 4] ascending
        diff = yb[:, None, :] - xb[cand]                     # [M, 4, 3]
        d = np.einsum("mgd,mgd->mg", diff, diff)
        j = np.argmin(d, axis=1)                             # ties -> lower n
        rows = np.arange(M)
        out_pmin[b] = d[rows, j].astype(np.float32)
        out_pidx[b] = cand[rows, j].astype(np.int32)

    return out_pmin, out_gmin, out_pidx, out_gidx


# revision 8
# speedup vs baseline: 2.2042x; 1.3263x over previous
"""Chamfer-loss min/argmin kernel for Trainium2 (8 NeuronCores).

Problem: preds [4, 8192, 3], gts [4, 8192, 3] fp32.
P[b, n, m] = ||gts[b,n]||^2 + ||preds[b,m]||^2 - 2 <gts[b,n], preds[b,m]>
Outputs: (min over n [4,8192], min over m [4,8192],
          argmin over n int32, argmin over m int32).

Sharding: 8 cores = 4 batches x 2 halves of the gts (n) axis. Each core
holds full preds for its batch and a 4096-row slice of gts.

Device kernel per core (both directions, roles swapped):
 - K=5 fp32 matmuls produce G = -P directly in PSUM (rows
   [2x0,2x1,2x2,-1,-rx] x [y0,y1,y2,ry,1]).  The reduced-axis operand is
   stored column-REVERSED on the host so forward streams scan descending
   original indices.
 - Groups of 1024 columns are processed in PAIRS: the scalar engine
   stages the even group PSUM->SBUF (its own PSUM port, otherwise idle),
   then ONE two-stream custom DVE op reads the odd group from PSUM (rd0)
   and the staged group from SBUF (rd1) at 2 elements/cycle total,
   computing accum = max stream-idx where max(s0,s1) == running-max.
   That is the first-occurrence (in original index space) argmax of
   G = argmin of P over the pair, up to a 1-bit which-stream ambiguity
   that the host resolves by evaluating both candidate distances.
 - Min values are reconstructed on the host from the candidate indices.
"""

import functools

import numpy as np

BS, N, M, D = 4, 8192, 8192, 3
NSL = N // 2  # gts rows per core
K = 5  # contraction: 3 coords + both norms
N_CORES = 8
GROUP = 1024      # columns per PSUM tile (2 banks)
PAIR = 2 * GROUP  # columns per paired DVE op
FMAX_NEG = np.float32(-3.4028234663852886e38)


def _register_pair_op():
    """Register the RUNMAX2_LAST_IDX_ANT custom DVE op (runtime append to
    dve_ops.OPS — the documented extension point; kernel.py must be
    self-contained so it cannot edit dve_ops.py)."""
    from concourse import dve_ops as dops
    from concourse.dve_spec import (
        Spec, Src0, Src1, MaxNeg, AluOp, eq, select, scan, Idx, maxx,
        lower as dve_lower, _has_src1,
    )
    from concourse.dve_uop import DveOpSpec

    name = "RUNMAX2_LAST_IDX_ANT"
    for o in dops.OPS:
        if o.name == name:
            return o

    mx = maxx(Src0, Src1)
    r = scan(AluOp.MAX, mx)
    body = select(eq(mx, r), Idx, MaxNeg)

    def _ref(in0, in1, s0, s1, imm2):
        x = np.asarray(in0, np.float32)
        y = np.asarray(in1, np.float32).reshape(x.shape)
        P = x.shape[0]
        xf, yf = x.reshape(P, -1), y.reshape(P, -1)
        m = np.maximum(xf, yf)
        rm = np.maximum.accumulate(m, axis=-1)
        idx = np.arange(xf.shape[-1], dtype=np.float32)
        out = np.where(m == rm, idx, FMAX_NEG)
        acc = out.max(axis=-1, keepdims=True)
        return out.reshape(x.shape).astype(np.float32), acc.astype(np.float32)

    spec = Spec(body=body, accum=AluOp.MAX, reference=_ref)
    shas = {}
    for ver in ("v3", "v4"):
        s = DveOpSpec(name=name, opcode=1, uops=dve_lower(spec, ver=ver),
                      rd1_en=_has_src1(spec))
        shas[ver] = s.sha(ver)
    op = dops.DveOp(name, spec, subdim=False, uops_sha=shas)
    dops.OPS.append(op)
    dops.CUSTOM_DVE_SPECS[name] = spec
    dops._SUB_OPCODE_FOR_NAME[name] = max(dops._SUB_OPCODE_FOR_NAME.values()) + 1
    return op


def _build_nc(nsl, m, reps=1):
    import contextlib

    import concourse.bacc as bacc
    import concourse.mybir as mybir
    import concourse.tile as tile

    op = _register_pair_op()
    f32 = mybir.dt.float32
    ident = mybir.ActivationFunctionType.Identity

    nc = bacc.Bacc("TRN2", target_bir_lowering=False, debug=False)

    ga = nc.declare_dram_parameter("ga", [2 * K, nsl], f32, isOutput=False)
    pa = nc.declare_dram_parameter("pa", [2 * K, m], f32, isOutput=False)
    np1 = m // PAIR    # pairs per dir-1 chunk
    np2 = nsl // PAIR  # pairs per dir-2 chunk
    gacc_o = nc.declare_dram_parameter("gacc", [128, nsl // 128 * np1], f32,
                                       isOutput=True)
    pacc_o = nc.declare_dram_parameter("pacc", [128, m // 128 * np2], f32,
                                       isOutput=True)

    with tile.TileContext(nc) as tc:
        with (
            tc.tile_pool(name="const", bufs=1) as const,
            tc.tile_pool(name="stg", bufs=3) as stg,
            tc.tile_pool(name="scr", bufs=1) as scr,
            tc.tile_pool(name="outs", bufs=1) as outs,
            tc.tile_pool(name="psA", bufs=2, space="PSUM") as psA,
            tc.tile_pool(name="psB", bufs=2, space="PSUM") as psB,
        ):
            # operands replicated into the 4 PE row groups (partitions 32j):
            # row groups 0,1 serve the staged (even) matmuls, 2,3 the direct
            ga_repL = const.tile([128, nsl], f32)
            ga_repR = const.tile([128, nsl], f32)
            pa_repR = const.tile([128, m], f32)
            pa_repL = const.tile([128, m], f32)
            for j in range(4):
                nc.sync.dma_start(ga_repL[32 * j : 32 * j + K, :], ga[0:K, :])
                nc.sync.dma_start(ga_repR[32 * j : 32 * j + K, :], ga[K : 2 * K, :])
                nc.sync.dma_start(pa_repR[32 * j : 32 * j + K, :], pa[0:K, :])
                nc.sync.dma_start(pa_repL[32 * j : 32 * j + K, :], pa[K : 2 * K, :])

            rep_loop = tc.For_i(0, reps, 1) if reps > 1 else contextlib.nullcontext()
            rep_loop.__enter__()

            gacc_sb = outs.tile([128, nsl // 128 * np1], f32)
            pacc_sb = outs.tile([128, m // 128 * np2], f32)
            scratch = scr.tile([128, GROUP], f32)

            def mm_group(pool, lhs_rep, rhs_rep, ci, col0, js, tag):
                pt = pool.tile([128, GROUP], f32, tag=tag)
                for i, j in enumerate(js):
                    lhsT = lhs_rep[32 * j : 32 * j + K, ci * 128 : (ci + 1) * 128]
                    c0 = col0 + i * 512
                    nc.tensor.matmul(
                        pt[:, i * 512 : (i + 1) * 512],
                        lhsT=lhsT,
                        rhs=rhs_rep[32 * j : 32 * j + K, c0 : c0 + 512],
                        start=True,
                        stop=True,
                        tile_position=(32 * j, 0),
                    )
                return pt

            def direction(n_chunks, lhs_rep, rhs_rep, rhs_len, acc_sb):
                n_pairs = rhs_len // PAIR
                for ci in range(n_chunks):
                    for p in range(n_pairs):
                        ptA = mm_group(psA, lhs_rep, rhs_rep, ci,
                                       p * PAIR, (0, 1), "ptA")
                        st = stg.tile([128, GROUP], f32, tag="st")
                        nc.scalar.activation(st[:], ptA[:], ident)
                        ptB = mm_group(psB, lhs_rep, rhs_rep, ci,
                                       p * PAIR + GROUP, (2, 3), "ptB")
                        nc.vector._custom_dve(
                            op,
                            out=scratch[:],
                            in0=ptB[:],
                            in1=st[:],
                            accum_out=acc_sb[:, ci * n_pairs + p :
                                             ci * n_pairs + p + 1],
                        )

            # per-gt rows: argmin over preds (final)
            direction(nsl // 128, ga_repL, pa_repR, m, gacc_sb)
            # per-pred rows: argmin over the gts slice (partial)
            direction(m // 128, pa_repL, ga_repR, nsl, pacc_sb)

            nc.sync.dma_start(gacc_o[:], gacc_sb[:])
            nc.sync.dma_start(pacc_o[:], pacc_sb[:])

            rep_loop.__exit__(None, None, None)
    nc.finalize()
    return nc


@functools.lru_cache(maxsize=None)
def _get_nc(nsl, m, reps=1):
    return _build_nc(nsl, m, reps)


def _augment(preds_b, gts_bh):
    """Operands for the negated K=5 scheme (G = -P in PSUM).

    ga rows 0-4 (dir-1 lhsT): [2x0, 2x1, 2x2, -1, -rx]
    ga rows 5-9 (dir-2 rhs):  [x0, x1, x2, rx, 1]   (column-REVERSED)
    pa rows 0-4 (dir-1 rhs):  [y0, y1, y2, ry, 1]   (column-REVERSED)
    pa rows 5-9 (dir-2 lhsT): [2y0, 2y1, 2y2, -1, -ry]
    G[n,m] = 2<x,y> - ry - rx = -P.
    """
    x = np.ascontiguousarray(gts_bh, dtype=np.float32)
    y = np.ascontiguousarray(preds_b, dtype=np.float32)
    nsl = x.shape[0]
    m = y.shape[0]
    rx = (x[:, 0] * x[:, 0] + x[:, 1] * x[:, 1] + x[:, 2] * x[:, 2]).astype(np.float32)
    ry = (y[:, 0] * y[:, 0] + y[:, 1] * y[:, 1] + y[:, 2] * y[:, 2]).astype(np.float32)
    ga = np.empty((2 * K, nsl), np.float32)
    ga[0:3] = np.float32(2.0) * x.T
    ga[3] = -1.0
    ga[4] = -rx
    ga[5:8] = x.T[:, ::-1]
    ga[8] = rx[::-1]
    ga[9] = 1.0
    pa = np.empty((2 * K, m), np.float32)
    pa[0:3] = y.T[:, ::-1]
    pa[3] = ry[::-1]
    pa[4] = 1.0
    pa[5:8] = np.float32(2.0) * y.T
    pa[8] = -1.0
    pa[9] = -ry
    return ga, pa


@functools.lru_cache(maxsize=None)
def _get_dispatcher(nsl, m, reps=1):
    """Build the SPMD PJRT dispatcher once and cache it (the stock
    run_bass_via_pjrt re-traces jax.jit on every call)."""
    import jax
    import numpy as _np
    from jax.sharding import Mesh, PartitionSpec
    from jax.experimental.shard_map import shard_map
    import concourse.mybir as mybir
    from concourse import bass2jax

    bass2jax.install_neuronx_cc_hook()
    nc = _get_nc(nsl, m, reps)

    partition_name = nc.partition_id_tensor.name if nc.partition_id_tensor else None
    in_names, out_names, out_avals, zero_outs = [], [], [], []
    for alloc in nc.m.functions[0].allocations:
        if not isinstance(alloc, mybir.MemoryLocationSet):
            continue
        name = alloc.memorylocations[0].name
        if alloc.kind == "ExternalInput":
            if name != partition_name:
                in_names.append(name)
        elif alloc.kind == "ExternalOutput":
            shape = tuple(alloc.tensor_shape)
            dtype = mybir.dt.np(alloc.dtype)
            out_names.append(name)
            out_avals.append(jax.core.ShapedArray(shape, dtype))
            zero_outs.append(_np.zeros(shape, dtype))
    n_params = len(in_names)
    n_outs = len(out_avals)
    all_in_names = list(in_names) + list(out_names)
    if partition_name is not None:
        all_in_names.append(partition_name)
    donate = tuple(range(n_params, n_params + n_outs))

    def _body(*args):
        operands = list(args)
        if partition_name is not None:
            operands.append(bass2jax.partition_id_tensor())
        outs = bass2jax._bass_exec_p.bind(
            *operands,
            out_avals=tuple(out_avals),
            in_names=tuple(all_in_names),
            out_names=tuple(out_names),
            lowering_input_output_aliases=(),
            sim_require_finite=True,
            sim_require_nnan=True,
            nc=nc,
        )
        return tuple(outs)

    devices = jax.devices()[:N_CORES]
    mesh = Mesh(np.asarray(devices), ("core",))
    in_specs = (PartitionSpec("core"),) * (n_params + n_outs)
    out_specs = (PartitionSpec("core"),) * n_outs
    sharded = jax.jit(
        shard_map(_body, mesh=mesh, in_specs=in_specs, out_specs=out_specs,
                  check_rep=False),
        donate_argnums=donate,
        keep_unused=True,
    )

    def dispatch(in_maps):
        concat_in = [
            np.concatenate([np.asarray(in_maps[c][nm]) for c in range(N_CORES)], axis=0)
            for nm in in_names
        ]
        concat_zeros = [
            np.zeros((N_CORES * z.shape[0], *z.shape[1:]), z.dtype) for z in zero_outs
        ]
        out_arrs = sharded(*concat_in, *concat_zeros)
        return [
            {nm: np.asarray(out_arrs[i]).reshape(N_CORES, *out_avals[i].shape)[c]
             for i, nm in enumerate(out_names)}
            for c in range(N_CORES)
        ]

    return dispatch


def _make_in_maps(preds, gts):
    in_maps = []
    for c in range(N_CORES):
        b, h = c // 2, c % 2
        ga, pa = _augment(preds[b], gts[b, h * NSL : (h + 1) * NSL])
        in_maps.append({"ga": ga, "pa": pa})
    return in_maps


def _decode_candidates(acc, n_rows, n_pairs, length):
    """acc: [128, n_rows//128 * n_pairs] stream indices -> candidate
    ORIGINAL column indices [n_rows, 2*n_pairs].

    The reduced-axis operand is column-reversed on the host, so flipped
    column q corresponds to original column (length-1-q).  Pair p covers
    flipped columns [p*PAIR, p*PAIR+GROUP) via the staged SBUF stream and
    [p*PAIR+GROUP, (p+1)*PAIR) via the PSUM stream; stream position k
    maps to both (pair ambiguity resolved by the caller on values).
    """
    k = np.rint(np.asarray(acc, np.float64)).astype(np.int64)
    k = k.reshape(128, n_rows // 128, n_pairs).transpose(1, 0, 2)
    k = k.reshape(n_rows, n_pairs)
    pbase = np.arange(n_pairs, dtype=np.int64)[None, :] * PAIR
    qS = pbase + k
    qB = pbase + GROUP + k
    cand = np.concatenate([qS, qB], axis=1)
    return (length - 1) - cand


def _pick(cand, d):
    """Lexicographic (distance, index) argmin per row: exact
    first-occurrence tie-break over the candidate set."""
    rows = np.arange(cand.shape[0])
    dmin = d.min(axis=1)
    masked = np.where(d == dmin[:, None], cand, np.int64(1) << 40)
    j = masked.min(axis=1)
    return dmin, j


def kernel(preds, gts, mask):
    preds = np.asarray(preds, dtype=np.float32)
    gts = np.asarray(gts, dtype=np.float32)

    results = _get_dispatcher(NSL, M)(_make_in_maps(preds, gts))

    out_pmin = np.empty((BS, M), np.float32)
    out_gmin = np.empty((BS, N), np.float32)
    out_pidx = np.empty((BS, M), np.int32)
    out_gidx = np.empty((BS, N), np.int32)

    np1 = M // PAIR
    np2 = NSL // PAIR

    for b in range(BS):
        r0, r1 = results[2 * b], results[2 * b + 1]
        xb = gts[b].astype(np.float64)
        yb = preds[b].astype(np.float64)

        # per-gt rows (min over preds): each half is final
        for h, r in ((0, r0), (1, r1)):
            cand = _decode_candidates(r["gacc"], NSL, np1, M)   # [NSL, 8]
            xrows = xb[h * NSL : (h + 1) * NSL]
            diff = xrows[:, None, :] - yb[cand]
            d = np.einsum("ngd,ngd->ng", diff, diff)
            dmin, j = _pick(cand, d)
            out_gmin[b, h * NSL : (h + 1) * NSL] = dmin.astype(np.float32)
            out_gidx[b, h * NSL : (h + 1) * NSL] = j.astype(np.int32)

        # per-pred rows: combine the two n-halves
        c0 = _decode_candidates(r0["pacc"], M, np2, NSL)        # [M, 4] half 0
        c1 = _decode_candidates(r1["pacc"], M, np2, NSL) + NSL  # [M, 4] half 1
        cand = np.concatenate([c0, c1], axis=1)                 # [M, 8]
        diff = yb[:, None, :] - xb[cand]
        d = np.einsum("mgd,mgd->mg", diff, diff)
        dmin, j = _pick(cand, d)
        out_pmin[b] = dmin.astype(np.float32)
        out_pidx[b] = j.astype(np.int32)

    return out_pmin, out_gmin, out_pidx, out_gidx
